# revision 33
# baseline (speedup 1.0000x reference)
"""MoE feed-forward (top-1 routing) on 8 TRN2 NeuronCores.

Sharding: tensor-parallel over D_FF on top of the expert dim. Core c holds
f-columns [c*512:(c+1)*512] of EVERY expert's w1/b1/w2 and processes the
full expert-sorted token stream, emitting a partial y; the host sums the 8
partials and adds b2. This makes the per-core work identical regardless of
how the router balances tokens (no expert-parallel load imbalance).

Host does the gate (tiny matmul) + dispatch/combine (the "all-to-all").
Device: y_part^T = w2s^T @ relu(w1s^T @ x^T + b1s), tokens kept in the
matmul free dimension throughout, so no on-device transposes. All weights
stay resident in SBUF as bf16.
"""

import os

import numpy as np
import ml_dtypes

import concourse.bass as bass
from concourse import bacc
import concourse.mybir as mybir
from concourse.tile import TileContext
from concourse.bass_utils import run_bass_kernel_spmd

P = 128
D_MODEL = 1024
D_FF = 4096
NUM_EXPERTS = 8
KD = D_MODEL // P   # 8  d-tiles
FH = D_FF // 8      # 512 f-columns per core
KH = FH // P        # 4  f-tiles per expert-slice

BF16 = mybir.dt.bfloat16
F32 = mybir.dt.float32


def _seg_chunks(C, first_small):
    """Split C into chunk widths <=512, avoiding tiny tails (<128)."""
    sizes = []
    rem = C
    if first_small and rem > 256:
        # small first chunk so the PE can start as soon as ~0.5MB has landed
        sizes.append(256)
        rem -= 256
    while rem > 576:
        sizes.append(512)
        rem -= 512
    if rem > 512:
        a = -(-(rem // 2) // 16) * 16
        sizes += [a, rem - a]
    elif rem:
        sizes.append(rem)
    return sizes


def _build(caps):
    nc = bacc.Bacc()
    CT = sum(caps)
    xT = nc.declare_dram_parameter("xT", [P, KD, CT], BF16, isOutput=False)
    w1 = nc.declare_dram_parameter("w1", [P, KD, D_FF], BF16, isOutput=False)
    b1 = nc.declare_dram_parameter("b1", [P, 8 * KH], F32, isOutput=False)
    w2 = nc.declare_dram_parameter("w2", [P, 8 * KH, D_MODEL], BF16, isOutput=False)
    out = nc.declare_dram_parameter("out", [P, KD, CT], BF16, isOutput=True)

    with TileContext(nc) as tc:
        with (
            tc.tile_pool(name="wpool", bufs=1) as wpool,
            tc.tile_pool(name="xpool", bufs=3) as xpool,
            tc.tile_pool(name="hpool", bufs=2) as hpool,
            tc.tile_pool(name="ypool", bufs=2) as ypool,
            tc.tile_pool(name="ps1", bufs=4, space="PSUM") as ps1pool,
            tc.tile_pool(name="ps2", bufs=4, space="PSUM") as ps2pool,
        ):
            # global chunk list: (expert_seg, global_col0, width)
            work = []
            off = 0
            for s in range(8):
                if caps[s] == 0:
                    continue
                sizes = _seg_chunks(caps[s], first_small=(len(work) == 0))
                c0 = 0
                for wdt in sizes:
                    work.append((s, off + c0, wdt))
                    c0 += wdt
                off += caps[s]

            # x for the first two chunks before any weight DMA; spread DMA
            # triggers across engine queues (issue is ~1us each, serialized
            # per queue).
            x_tiles = {}
            for wi, (s, g0, cw) in enumerate(work[:2]):
                x_sb = xpool.tile([P, KD, 512], BF16, tag="x")
                nc.sync.dma_start(x_sb[:, :, :cw], xT[:, :, g0:g0 + cw])
                x_tiles[wi] = x_sb

            b1_sb = wpool.tile([P, 8 * KH], F32, tag="b1")
            nc.scalar.dma_start(b1_sb[:], b1[:])


            # Resident weights: per expert-segment slices, interleaved in
            # the order compute consumes them (w1_s before w2_s). Each 1MB
            # slice is split across two DMA queues (a single queue moves
            # ~45GB/s); triggers go on gpsimd, which is otherwise idle.
            w1_t, w2_t = [], []
            for s in range(8):
                t1 = wpool.tile([P, KD, FH], BF16, tag=f"w1_{s}")
                if s == 0:
                    nc.gpsimd.dma_start(t1[:, :, :128], w1[:, :, :128])
                    nc.gpsimd.dma_start(t1[:, :, 128:256], w1[:, :, 128:256])
                    nc.gpsimd.dma_start(t1[:, :, 256:], w1[:, :, 256:FH])
                else:
                    h = FH // 2
                    o = s * FH
                    nc.gpsimd.dma_start(t1[:, :, :h], w1[:, :, o:o + h])
                    nc.gpsimd.dma_start(t1[:, :, h:], w1[:, :, o + h:o + FH])
                w1_t.append(t1)
                t2 = wpool.tile([P, KH, D_MODEL], BF16, tag=f"w2_{s}")
                nc.gpsimd.dma_start(t2[:, :2], w2[:, s * KH:s * KH + 2])
                nc.gpsimd.dma_start(t2[:, 2:], w2[:, s * KH + 2:(s + 1) * KH])
                w2_t.append(t2)

            for wi, (s, g0, cw) in enumerate(work):
                if wi in x_tiles:
                    x_sb = x_tiles[wi]
                else:
                    x_sb = xpool.tile([P, KD, 512], BF16, tag="x")
                    nc.sync.dma_start(x_sb[:, :, :cw], xT[:, :, g0:g0 + cw])

                h_sb = hpool.tile([P, KH, 512], BF16, tag="h")
                # FFN1: H^T[fo] = relu(w1s[:, fo]^T @ x^T + b1s[fo])
                for fo in range(KH):
                    ps = ps1pool.tile([P, 512], F32, tag="ps1")
                    for ko in range(KD):
                        nc.tensor.matmul(
                            ps[:, :cw],
                            w1_t[s][:, ko, fo * P:(fo + 1) * P],
                            x_sb[:, ko, :cw],
                            start=(ko == 0),
                            stop=(ko == KD - 1),
                        )
                    nc.scalar.activation(
                        h_sb[:, fo, :cw],
                        ps[:, :cw],
                        mybir.ActivationFunctionType.Relu,
                        bias=b1_sb[:, s * KH + fo:s * KH + fo + 1],
                    )
                # FFN2 partial: y^T[do] = w2s[:, do]^T @ H^T  (b2 on host)
                last = wi == len(work) - 1
                y_sb = ypool.tile([P, KD, 512], BF16, tag="y")
                for do in range(KD):
                    ps2 = ps2pool.tile([P, 512], F32, tag="ps2")
                    for fo in range(KH):
                        nc.tensor.matmul(
                            ps2[:, :cw],
                            w2_t[s][:, fo, do * P:(do + 1) * P],
                            h_sb[:, fo, :cw],
                            start=(fo == 0),
                            stop=(fo == KH - 1),
                        )
                    nc.vector.tensor_copy(y_sb[:, do, :cw], ps2[:, :cw])
                    if last:
                        # stream the tail out per do-group to shorten the drain
                        nc.sync.dma_start(out[:, do, g0:g0 + cw], y_sb[:, do, :cw])
                if not last:
                    nc.sync.dma_start(out[:, :, g0:g0 + cw], y_sb[:, :, :cw])
    nc.compile()
    return nc


_NC_CACHE = {}
LAST_EXEC_NS = None


def _get_nc(caps):
    if caps not in _NC_CACHE:
        _NC_CACHE[caps] = _build(caps)
    return _NC_CACHE[caps]


def _part3(a, kd):
    # [kd*P, cols...] -> [P, kd, cols] partition-inner layout
    return np.ascontiguousarray(
        a.reshape(kd, P, a.shape[1]).transpose(1, 0, 2))


def kernel(x, gate_w, gate_b, expert_bias, w1, b1, w2, b2):
    global LAST_EXEC_NS
    B, S, D = x.shape
    xf = np.ascontiguousarray(x.reshape(-1, D)).astype(np.float32)

    logits = xf @ gate_w.T.astype(np.float32) + (gate_b + expert_bias)
    top = logits.argmax(-1)

    counts = np.bincount(top, minlength=NUM_EXPERTS)
    caps = tuple(int(-(-c // 16) * 16) for c in counts)
    CT = sum(caps)

    # Expert-sorted padded token stream, shared by all cores.
    idx_lists = []
    xg = np.zeros((CT, D), np.float32)
    off = 0
    offs = []
    for e in range(NUM_EXPERTS):
        ids = np.nonzero(top == e)[0]
        idx_lists.append(ids)
        offs.append(off)
        xg[off:off + len(ids)] = xf[ids]
        off += caps[e]
    xT = _part3(np.ascontiguousarray(xg.T).astype(ml_dtypes.bfloat16), KD)

    w1f = np.asarray(w1, np.float32)
    w2f = np.asarray(w2, np.float32)
    b1f = np.asarray(b1, np.float32)

    in_maps = []
    for c in range(NUM_EXPERTS):
        fs = slice(c * FH, (c + 1) * FH)
        # pack every expert's f-slice side by side
        w1c = np.concatenate([w1f[e][:, fs] for e in range(NUM_EXPERTS)],
                             axis=1).astype(ml_dtypes.bfloat16)   # [D, 8*FH]
        w2c = np.concatenate([w2f[e][fs, :] for e in range(NUM_EXPERTS)],
                             axis=0).astype(ml_dtypes.bfloat16)   # [8*FH, D]
        b1c = np.stack([b1f[e][fs] for e in range(NUM_EXPERTS)])  # [8, FH]
        in_maps.append({
            "xT": xT,
            "w1": _part3(w1c, KD),
            "w2": _part3(w2c, 8 * KH),
            "b1": np.ascontiguousarray(b1c.reshape(8 * KH, P).T),
        })

    nc = _get_nc(caps)
    res = None
    for attempt in range(3):
        try:
            res = run_bass_kernel_spmd(nc, in_maps, list(range(NUM_EXPERTS)))
            break
        except Exception:
            # rare transient NRT_EXEC_UNIT_UNRECOVERABLE from the runtime;
            # a straight retry has been observed to succeed
            if attempt == 2:
                raise
            import time
            time.sleep(5)
    LAST_EXEC_NS = res.exec_time_ns

    acc = np.zeros((P, KD, CT), np.float32)
    for c in range(NUM_EXPERTS):
        acc += np.asarray(res.results[c]["out"]).astype(np.float32)
    yg = acc.transpose(1, 0, 2).reshape(D, CT).T   # [CT, D]

    out = np.zeros_like(xf)
    for e in range(NUM_EXPERTS):
        ids = idx_lists[e]
        if len(ids):
            out[ids] = yg[offs[e]:offs[e] + len(ids)] + b2[e]
    return out.reshape(B, S, D)
